# revision 15
# baseline (speedup 1.0000x reference)
"""Leaky-integrator linear recurrence kernel for Trainium2.

u_t = TAU * u_{t-1} + x_t along the last (time) axis of x[32, 1024, 2048] f32.

Strategy: data-parallel across 8 NeuronCores (4 batches each), 16-bit HBM
traffic (the 2e-2 tolerance dwarfs bf16 quantization), and the recurrence
computed on the Tensor engine as a triangular matmul with the carry folded
into the contraction. In a host-transposed layout xt[time, rows], time is
cut into blocks of 127 steps. The moving tile of block b is

    partition 0      : u[t0-1, :]   (carry row; zeros for block 0 via a
                                     host-prepended zero row in xt)
    partitions 1..127: x[t0 .. t0+126, :]

and ONE constant stationary S[k, m] (S[0, m] = TAU^(m+1) carry column,
S[1+j, m] = TAU^(m-j) for j <= m) yields the exact block output
u[t0 .. t0+126] in a single FD=512 pass per PSUM chunk:

    u[t0+m] = TAU^(m+1) u[t0-1] + sum_j TAU^(m-j) x[t0+j]

Because every matmul shares the same stationary, all but the first
LDWEIGHTS are redundant; _dedup_ldweights() removes them (~100 ns of PE
each). This halves Tensor-engine work versus the two-matmul (cross-block
band + triangular band) formulation and takes PE off the critical path.
The carry row travels between consecutive blocks' SBUF tiles via tiny
per-chunk SWDGE DMAs (gpsimd queue — its own rings, so they never queue
behind megabyte slab/output transfers). A 16-step remainder block with its
own small stationary covers 2048 = 16*127 + 16.

Engine assignment: Sync issues input DMAs, Scalar issues output DMAs (two
HWDGE rings — input prefetch never head-of-line blocks behind output
drain), GpSimd issues the carry DMAs, Tensor does the matmuls, and the
PSUM f32 -> SBUF bf16 downcast copies alternate between Vector and Scalar.

The walrus build in this container allows at most ONE embedded sync-wait
per engine instruction (two on EventSemaphore); Tile's wait assignment can
attach several. _split_excess_waits() hoists the extras onto standalone
EventSemaphore instructions inserted immediately before, on the same
engine — conservative but correct, since every awaited semaphore's
producer precedes the waiter in the scheduled program order.
"""

import numpy as np
import ml_dtypes

import concourse.bass as bass
import concourse.mybir as mybir
from concourse.bass_utils import run_bass_kernel_spmd
from concourse.tile import TileContext

TAU = 0.9
B, F, T = 32, 1024, 2048
N_CORES = 8
B_PER_CORE = B // N_CORES          # 4
ROWS = B_PER_CORE * F              # 4096 independent recurrences per core
P = 128
BLK = P - 1                        # 127 time steps per full block
N_BLK = T // BLK                   # 16 full blocks
REM = T - N_BLK * BLK              # 16-step remainder
CHUNK = 512                        # PSUM bank width (f32)
N_CHUNK = ROWS // CHUNK            # 8

NP_DT = ml_dtypes.bfloat16
MYBIR_DT = mybir.dt.bfloat16

_nc_cache = None
_coef_cache = None
last_results = None  # BassKernelResults from the most recent run (for test.py)


def _split_excess_waits(nc: bass.Bass) -> None:
    for fn in nc.m.functions:
        for blk in fn.blocks:
            out = []
            changed = False
            for inst in blk.instructions:
                si = inst.sync_info
                waits = list(si.on_wait) if si is not None else []
                cap = 2 if inst.opcode == "EventSemaphore" else 1
                if len(waits) <= cap:
                    out.append(inst)
                    continue
                changed = True
                # On DMAs keep a queue-ordering (DMAHW*) wait embedded so
                # queue-level throttling stays at the queue; otherwise keep
                # the last wait.
                keep_idx = len(waits) - 1
                if inst.opcode == "DMACopy":
                    for k, w in enumerate(waits):
                        if (w.ant_name or "").startswith("DMA"):
                            keep_idx = k
                            break
                rest = [w for j, w in enumerate(waits) if j != keep_idx]
                for j in range(0, len(rest), 2):
                    out.append(
                        mybir.InstEventSemaphore(
                            name=f"{inst.name}-xw{j}",
                            opcode="EventSemaphore",
                            engine=inst.engine,
                            debug=inst.debug,
                            sync_info=mybir.SyncInfo(
                                on_wait=rest[j : j + 2], on_update=[]
                            ),
                        )
                    )
                inst.sync_info = mybir.SyncInfo(
                    on_wait=[waits[keep_idx]], on_update=list(si.on_update)
                )
                out.append(inst)
            if changed:
                blk.instructions = out


def _dedup_ldweights(nc: bass.Bass) -> None:
    """Drop PE weight reloads that reload the already-loaded stationary.

    tile_legalize splits every matmul into InstLdweights + a
    non-self-loading InstMatmult. Matmult does not clobber the PE weight
    array, so consecutive Ldweights with an identical weights AP are
    redundant — all but the first can go (~100 ns of PE time each). A
    redundant Ldweights that carries semaphore waits/updates is replaced
    by an EventSemaphore on the same engine so the synchronization is
    preserved; any other PE instruction resets the tracked signature.
    """
    for fn in nc.m.functions:
        for blk in fn.blocks:
            out = []
            changed = False
            last_sig = None
            for inst in blk.instructions:
                if inst.opcode == "Matmult":
                    out.append(inst)
                    continue
                if inst.opcode != "Ldweights":
                    if inst.engine == mybir.EngineType.PE and inst.opcode not in (
                        "EventSemaphore",
                    ):
                        last_sig = None
                    out.append(inst)
                    continue
                a = inst.ins[0]
                sig = (a.memref, a.offset, str(a.ap), str(a.dtype))
                if sig != last_sig:
                    last_sig = sig
                    out.append(inst)
                    continue
                changed = True
                si = inst.sync_info
                waits = list(si.on_wait) if si is not None else []
                upds = list(si.on_update) if si is not None else []
                if waits or upds:
                    for j in range(0, max(len(waits), 1), 2):
                        out.append(
                            mybir.InstEventSemaphore(
                                name=f"{inst.name}-lw{j}",
                                opcode="EventSemaphore",
                                engine=inst.engine,
                                debug=inst.debug,
                                sync_info=mybir.SyncInfo(
                                    on_wait=waits[j : j + 2],
                                    on_update=upds if j == 0 else [],
                                ),
                            )
                        )
            if changed:
                blk.instructions = out


def _coef() -> np.ndarray:
    # [P, BLK + REM] = [S | S2] packed side by side (one SBUF tile/DMA).
    # S  [128, 127]: S[0, m]  = TAU^(m+1); S[1+j, m] = TAU^(m-j) for j<=m.
    # S2 [17, 16] in rows 0..16 of the last REM columns: same structure.
    def stat(n):
        j = np.arange(n)[None, :]           # output row m
        k = np.arange(n)[:, None]           # x offset j
        tri = np.where(k <= j, TAU ** np.maximum(j - k, 0).astype(np.float64), 0.0)
        top = TAU ** (np.arange(n, dtype=np.float64) + 1.0)[None, :]
        return np.vstack([top, tri])        # [n+1, n]

    c = np.zeros((P, BLK + REM), dtype=np.float64)
    c[0 : P, 0:BLK] = stat(BLK)
    c[0 : REM + 1, BLK : BLK + REM] = stat(REM)
    return np.ascontiguousarray(c.astype(NP_DT))


def _build() -> bass.Bass:
    nc = bass.Bass()
    # xt row 0 is a host-written zero row: block 0's carry. Row 1+t = x[t].
    xt = nc.dram_tensor("xt", [T + 1, ROWS], MYBIR_DT, kind="ExternalInput")
    coef = nc.dram_tensor("coef", [P, BLK + REM], MYBIR_DT, kind="ExternalInput")
    yt = nc.dram_tensor("yt", [T, ROWS], MYBIR_DT, kind="ExternalOutput")

    with TileContext(nc) as tc:
        with (
            tc.tile_pool(name="const", bufs=1) as cpool,
            tc.tile_pool(name="in", bufs=8) as ipool,
            tc.tile_pool(name="out", bufs=4) as opool,
            tc.tile_pool(name="psum", bufs=8, space="PSUM") as ppool,
        ):
            cf = cpool.tile([P, BLK + REM], MYBIR_DT)
            nc.scalar.dma_start(out=cf[:], in_=coef[:])
            cS = cf[:, 0:BLK]                      # [128, 127]
            cS2 = cf[0 : REM + 1, BLK : BLK + REM]  # [17, 16]

            prev_ut = None
            for b in range(N_BLK + 1):
                last = b == N_BLK
                kp = (REM + 1) if last else P      # moving partitions
                om = REM if last else BLK          # output rows
                t0 = b * BLK
                s = ipool.tile([P, ROWS], MYBIR_DT)
                if b == 0:
                    # carry row 0 comes from the host zero row: one full DMA
                    nc.sync.dma_start(out=s[0:P, :], in_=xt[0:P, :])
                else:
                    nc.sync.dma_start(
                        out=s[1:kp, :], in_=xt[t0 + 1 : t0 + kp, :]
                    )
                utile = opool.tile([P, ROWS], MYBIR_DT)
                for c in range(N_CHUNK):
                    sl = slice(c * CHUNK, (c + 1) * CHUNK)
                    if b > 0:
                        # inject the carry u[t0-1] into moving partition 0
                        # (HWDGE on the otherwise-idle Sync engine: SWDGE's
                        # per-DMA latency was ~8 us and serialized the chain)
                        nc.sync.dma_start(
                            out=s[0:1, sl], in_=prev_ut[BLK - 1 : BLK, sl]
                        )
                    pt = ppool.tile([P, CHUNK], mybir.dt.float32)
                    nc.tensor.matmul(
                        pt[0:om, :],
                        lhsT=cS2 if last else cS,
                        rhs=s[0:kp, sl],
                        start=True,
                        stop=True,
                    )
                    if c % 2 == 0:
                        nc.vector.tensor_copy(utile[0:om, sl], pt[0:om, :])
                    else:
                        nc.scalar.copy(utile[0:om, sl], pt[0:om, :])
                nc.scalar.dma_start(out=yt[t0 : t0 + om, :], in_=utile[0:om, :])
                prev_ut = utile

    _dedup_ldweights(nc)
    _split_excess_waits(nc)
    return nc


def kernel(x: np.ndarray, **_unused) -> np.ndarray:
    global _nc_cache, _coef_cache, last_results
    if _nc_cache is None:
        _nc_cache = _build()
        _coef_cache = _coef()
    nc = _nc_cache

    x = np.asarray(x)
    assert x.shape == (B, F, T), x.shape
    x16 = np.ascontiguousarray(x.reshape(N_CORES, ROWS, T), dtype=NP_DT)
    in_maps = []
    for c in range(N_CORES):
        xt = np.zeros((T + 1, ROWS), dtype=NP_DT)
        xt[1:] = x16[c].T
        in_maps.append({"xt": xt, "coef": _coef_cache})
    last_results = run_bass_kernel_spmd(
        nc, in_maps, core_ids=list(range(N_CORES))
    )
    out = np.concatenate(
        [
            r["yt"].T.astype(np.float32).reshape(B_PER_CORE, F, T)
            for r in last_results.results
        ],
        axis=0,
    )
    return out


# revision 20
# speedup vs baseline: 7.5474x; 7.5474x over previous
"""Leaky-integrator linear recurrence kernel for Trainium2.

u_t = TAU * u_{t-1} + x_t along the last (time) axis of x[32, 1024, 2048] f32.

Strategy: data-parallel across 8 NeuronCores (4 batches each), 16-bit HBM
traffic (the 2e-2 tolerance dwarfs bf16 quantization), and the recurrence
computed on the Tensor engine as a triangular matmul with the carry folded
into the contraction. In a host-transposed layout xt[time, rows], time is
cut into blocks of 127 steps. The moving tile of block b is

    partition 0      : u[t0-1, :]   (carry row; zeros for block 0 via a
                                     host-prepended zero row in xt)
    partitions 1..127: x[t0 .. t0+126, :]

and ONE constant stationary S[k, m] (S[0, m] = TAU^(m+1) carry column,
S[1+j, m] = TAU^(m-j) for j <= m) yields the exact block output
u[t0 .. t0+126] in a single FD=512 pass per PSUM chunk:

    u[t0+m] = TAU^(m+1) u[t0-1] + sum_j TAU^(m-j) x[t0+j]

Because every matmul shares the same stationary, all but the first
LDWEIGHTS are redundant; _dedup_ldweights() removes them (~100 ns of PE
each). This halves Tensor-engine work versus the two-matmul (cross-block
band + triangular band) formulation and takes PE off the critical path.
The carry row travels between consecutive blocks' SBUF tiles via tiny
per-chunk SWDGE DMAs (gpsimd queue — its own rings, so they never queue
behind megabyte slab/output transfers). A 16-step remainder block with its
own small stationary covers 2048 = 16*127 + 16.

Engine assignment: Sync issues input DMAs, Scalar issues output DMAs (two
HWDGE rings — input prefetch never head-of-line blocks behind output
drain), GpSimd issues the carry DMAs, Tensor does the matmuls, and the
PSUM f32 -> SBUF bf16 downcast copies alternate between Vector and Scalar.

The walrus build in this container allows at most ONE embedded sync-wait
per engine instruction (two on EventSemaphore); Tile's wait assignment can
attach several. _split_excess_waits() hoists the extras onto standalone
EventSemaphore instructions inserted immediately before, on the same
engine — conservative but correct, since every awaited semaphore's
producer precedes the waiter in the scheduled program order.
"""

import numpy as np
import ml_dtypes

import concourse.bass as bass
import concourse.mybir as mybir
from concourse.bass_utils import run_bass_kernel_spmd
from concourse.tile import TileContext

TAU = 0.9
B, F, T = 32, 1024, 2048
N_CORES = 8
B_PER_CORE = B // N_CORES          # 4
ROWS = B_PER_CORE * F              # 4096 independent recurrences per core
P = 128
BLK = P - 1                        # 127 time steps per full block
N_BLK = T // BLK                   # 16 full blocks
REM = T - N_BLK * BLK              # 16-step remainder
CHUNK = 512                        # PSUM bank width (f32)
N_CHUNK = ROWS // CHUNK            # 8

NP_DT = ml_dtypes.bfloat16
MYBIR_DT = mybir.dt.bfloat16

_nc_cache = None
_coef_cache = None
last_results = None  # BassKernelResults from the most recent run (for test.py)


def _split_excess_waits(nc: bass.Bass) -> None:
    for fn in nc.m.functions:
        for blk in fn.blocks:
            out = []
            changed = False
            for inst in blk.instructions:
                si = inst.sync_info
                waits = list(si.on_wait) if si is not None else []
                cap = 2 if inst.opcode == "EventSemaphore" else 1
                if len(waits) <= cap:
                    out.append(inst)
                    continue
                changed = True
                # On DMAs keep a queue-ordering (DMAHW*) wait embedded so
                # queue-level throttling stays at the queue; otherwise keep
                # the last wait.
                keep_idx = len(waits) - 1
                if inst.opcode == "DMACopy":
                    for k, w in enumerate(waits):
                        if (w.ant_name or "").startswith("DMA"):
                            keep_idx = k
                            break
                rest = [w for j, w in enumerate(waits) if j != keep_idx]
                for j in range(0, len(rest), 2):
                    out.append(
                        mybir.InstEventSemaphore(
                            name=f"{inst.name}-xw{j}",
                            opcode="EventSemaphore",
                            engine=inst.engine,
                            debug=inst.debug,
                            sync_info=mybir.SyncInfo(
                                on_wait=rest[j : j + 2], on_update=[]
                            ),
                        )
                    )
                inst.sync_info = mybir.SyncInfo(
                    on_wait=[waits[keep_idx]], on_update=list(si.on_update)
                )
                out.append(inst)
            if changed:
                blk.instructions = out


def _dedup_ldweights(nc: bass.Bass) -> None:
    """Drop PE weight reloads that reload the already-loaded stationary.

    tile_legalize splits every matmul into InstLdweights + a
    non-self-loading InstMatmult. Matmult does not clobber the PE weight
    array, so consecutive Ldweights with an identical weights AP are
    redundant — all but the first can go (~100 ns of PE time each). A
    redundant Ldweights that carries semaphore waits/updates is replaced
    by an EventSemaphore on the same engine so the synchronization is
    preserved; any other PE instruction resets the tracked signature.
    """
    for fn in nc.m.functions:
        for blk in fn.blocks:
            out = []
            changed = False
            last_sig = None
            for inst in blk.instructions:
                if inst.opcode == "Matmult":
                    out.append(inst)
                    continue
                if inst.opcode != "Ldweights":
                    if inst.engine == mybir.EngineType.PE and inst.opcode not in (
                        "EventSemaphore",
                    ):
                        last_sig = None
                    out.append(inst)
                    continue
                a = inst.ins[0]
                sig = (a.memref, a.offset, str(a.ap), str(a.dtype))
                if sig != last_sig:
                    last_sig = sig
                    out.append(inst)
                    continue
                changed = True
                si = inst.sync_info
                waits = list(si.on_wait) if si is not None else []
                upds = list(si.on_update) if si is not None else []
                if waits or upds:
                    for j in range(0, max(len(waits), 1), 2):
                        out.append(
                            mybir.InstEventSemaphore(
                                name=f"{inst.name}-lw{j}",
                                opcode="EventSemaphore",
                                engine=inst.engine,
                                debug=inst.debug,
                                sync_info=mybir.SyncInfo(
                                    on_wait=waits[j : j + 2],
                                    on_update=upds if j == 0 else [],
                                ),
                            )
                        )
            if changed:
                blk.instructions = out


def _coef() -> np.ndarray:
    # [P, 2P] = [S | S2] packed side by side (one SBUF tile/DMA). Both
    # stationaries are full 128x128 (zero-padded output columns) so every
    # matmul writes all 128 PSUM rows and every DMA is 128-partition
    # aligned — partial-partition APs defeat balance_dma_aps and serialize
    # the transfer onto a single DMA engine.
    # Moving layout: x offsets on partitions 0..126 (S) / 111..126 (S2),
    # carry u[t0-1] on partition 127.
    # S [k, m] = TAU^(m-k) for k<=m<BLK;  S [127, m] = TAU^(m+1)
    # S2[111+j, m] = TAU^(m-j) for j<=m<REM; S2[127, m] = TAU^(m+1)
    def tri(n):
        j = np.arange(n)[None, :]           # output row m
        k = np.arange(n)[:, None]           # x offset
        return np.where(k <= j, TAU ** np.maximum(j - k, 0).astype(np.float64), 0.0)

    c = np.zeros((P, 2 * P), dtype=np.float64)
    c[0:BLK, 0:BLK] = tri(BLK)
    c[P - 1, 0:BLK] = TAU ** (np.arange(BLK, dtype=np.float64) + 1.0)
    c[P - 1 - REM : P - 1, P : P + REM] = tri(REM)
    c[P - 1, P : P + REM] = TAU ** (np.arange(REM, dtype=np.float64) + 1.0)
    return np.ascontiguousarray(c.astype(NP_DT))


def _build() -> bass.Bass:
    nc = bass.Bass()
    # xt rows 0..T-1 = x (host-transposed); row T is a host-written zero row
    # (block 0's carry source, and junk-pad for the remainder block's load).
    xt = nc.dram_tensor("xt", [T + 1, ROWS], MYBIR_DT, kind="ExternalInput")
    coef = nc.dram_tensor("coef", [P, 2 * P], MYBIR_DT, kind="ExternalInput")
    # Each block writes a full 128-row region (127 u rows + 1 zero row) so
    # every output DMA is 128-partition-aligned; the host strips the pad.
    yt = nc.dram_tensor("yt", [(N_BLK + 1) * P, ROWS], MYBIR_DT, kind="ExternalOutput")

    with TileContext(nc) as tc:
        with (
            tc.tile_pool(name="const", bufs=1) as cpool,
            tc.tile_pool(name="in", bufs=8) as ipool,
            tc.tile_pool(name="out", bufs=4) as opool,
            tc.tile_pool(name="psum", bufs=8, space="PSUM") as ppool,
        ):
            cf = cpool.tile([P, 2 * P], MYBIR_DT)
            nc.scalar.dma_start(out=cf[:], in_=coef[:])
            cS = cf[:, 0:P]                      # [128, 128]
            cS2 = cf[:, P : 2 * P]               # [128, 128]

            prev_ut = None
            for b in range(N_BLK + 1):
                last = b == N_BLK
                t0 = b * BLK
                # Full-width 128-row load (keeps the DMA split across all 16
                # engines); partition 127 gets a junk x row, overwritten by
                # the carry DMA below. The remainder block loads the last
                # 128 xt rows so its 16 x rows land on partitions 111..126.
                lo = (T + 1 - P) if last else t0
                s = ipool.tile([P, ROWS], MYBIR_DT)
                nc.sync.dma_start(out=s[:], in_=xt[lo : lo + P, :])
                utile = opool.tile([P, ROWS], MYBIR_DT)
                for c in range(N_CHUNK):
                    sl = slice(c * CHUNK, (c + 1) * CHUNK)
                    # inject the carry u[t0-1] into moving partition 127
                    # (the zeros row of xt for block 0)
                    if b == 0:
                        if c == 0:
                            nc.sync.dma_start(
                                out=s[P - 1 : P, :], in_=xt[T : T + 1, :]
                            )
                    else:
                        nc.sync.dma_start(
                            out=s[P - 1 : P, sl],
                            in_=prev_ut[BLK - 1 : BLK, sl],
                        )
                    pt = ppool.tile([P, CHUNK], mybir.dt.float32)
                    nc.tensor.matmul(
                        pt[:],
                        lhsT=cS2 if last else cS,
                        rhs=s[:, sl],
                        start=True,
                        stop=True,
                    )
                    if c % 2 == 0:
                        nc.vector.tensor_copy(utile[:, sl], pt[:])
                    else:
                        nc.scalar.copy(utile[:, sl], pt[:])
                nc.scalar.dma_start(out=yt[b * P : (b + 1) * P, :], in_=utile[:])
                prev_ut = utile

    _dedup_ldweights(nc)
    _split_excess_waits(nc)
    return nc


def kernel(x: np.ndarray, **_unused) -> np.ndarray:
    global _nc_cache, _coef_cache, last_results
    if _nc_cache is None:
        _nc_cache = _build()
        _coef_cache = _coef()
    nc = _nc_cache

    x = np.asarray(x)
    assert x.shape == (B, F, T), x.shape
    x16 = np.ascontiguousarray(x.reshape(N_CORES, ROWS, T), dtype=NP_DT)
    in_maps = []
    for c in range(N_CORES):
        xt = np.zeros((T + 1, ROWS), dtype=NP_DT)
        xt[:T] = x16[c].T
        in_maps.append({"xt": xt, "coef": _coef_cache})
    last_results = run_bass_kernel_spmd(
        nc, in_maps, core_ids=list(range(N_CORES))
    )
    # yt blocks are 128 rows each: 127 u rows + 1 pad row (REM valid rows
    # for the last); strip the pad and reassemble the [T, ROWS] result.
    outs = []
    for r in last_results.results:
        ytp = r["yt"]
        rows = [ytp[b * P : b * P + BLK] for b in range(N_BLK)]
        rows.append(ytp[N_BLK * P : N_BLK * P + REM])
        u = np.concatenate(rows, axis=0)          # [T, ROWS]
        outs.append(u.T.astype(np.float32).reshape(B_PER_CORE, F, T))
    return np.concatenate(outs, axis=0)


# revision 21
# speedup vs baseline: 13.2344x; 1.7535x over previous
"""Leaky-integrator linear recurrence kernel for Trainium2.

u_t = TAU * u_{t-1} + x_t along the last (time) axis of x[32, 1024, 2048] f32.

Strategy: data-parallel across 8 NeuronCores (4 batches each). The problem is
memory-bound, so HBM traffic is halved by moving data as 16-bit floats (the
2e-2 tolerance dwarfs the quantization error). The recurrence is computed on
the Tensor engine as a *banded matmul*: since TAU^129 < 2e-6, u_t is (to
float precision) a windowed sum u_t = sum_{s=t-255..t} TAU^(t-s) x_s. In a
host-transposed layout xt[time, rows], each 128-step output block i is

    u[i*128+m, r] = sum_{k} A[k, m] * xt[(i-1)*128+k, r]   (cross-block band)
                  + sum_{k} B[k, m] * xt[i*128+k, r]       (triangular band)

with A[k, m] = TAU^(m+128-k), B[k, m] = TAU^(m-k) for k<=m else 0 — two
accumulating 128x128-stationary matmuls per PSUM chunk (block 0 skips A).

Engine assignment: Sync issues input DMAs, Scalar issues output DMAs (two
HWDGE rings, so input prefetch never head-of-line blocks behind output
drain), Tensor does the matmuls, and the PSUM f32 -> SBUF 16-bit downcast
copies are split between Vector and Scalar (each ~46 us; a single engine
at ~92 us would sit on the critical path).

The walrus build in this container allows at most ONE embedded sync-wait
per engine instruction (two on EventSemaphore); Tile's wait assignment can
attach several. _split_excess_waits() hoists the extras onto standalone
EventSemaphore instructions inserted immediately before, on the same
engine — conservative but correct, since every awaited semaphore's
producer precedes the waiter in the scheduled program order.
"""

import numpy as np
import ml_dtypes

import concourse.bass as bass
import concourse.mybir as mybir
from concourse.bass_utils import run_bass_kernel_spmd
from concourse.tile import TileContext

TAU = 0.9
B, F, T = 32, 1024, 2048
N_CORES = 8
B_PER_CORE = B // N_CORES          # 4
ROWS = B_PER_CORE * F              # 4096 independent recurrences per core
P = 128
N_BLK = T // P                     # 16 time-blocks (slabs) per core
CHUNK = 512                        # PSUM bank width (f32)
N_CHUNK = ROWS // CHUNK            # 8

NP_DT = ml_dtypes.bfloat16
MYBIR_DT = mybir.dt.bfloat16

_nc_cache = None
_coef_cache = None
last_results = None  # BassKernelResults from the most recent run (for test.py)


def _split_excess_waits(nc: bass.Bass) -> None:
    for fn in nc.m.functions:
        for blk in fn.blocks:
            out = []
            changed = False
            for inst in blk.instructions:
                si = inst.sync_info
                waits = list(si.on_wait) if si is not None else []
                cap = 2 if inst.opcode == "EventSemaphore" else 1
                if len(waits) <= cap:
                    out.append(inst)
                    continue
                changed = True
                # On DMAs keep a queue-ordering (DMAHW*) wait embedded so
                # queue-level throttling stays at the queue; otherwise keep
                # the last wait.
                keep_idx = len(waits) - 1
                if inst.opcode == "DMACopy":
                    for k, w in enumerate(waits):
                        if (w.ant_name or "").startswith("DMA"):
                            keep_idx = k
                            break
                rest = [w for j, w in enumerate(waits) if j != keep_idx]
                for j in range(0, len(rest), 2):
                    out.append(
                        mybir.InstEventSemaphore(
                            name=f"{inst.name}-xw{j}",
                            opcode="EventSemaphore",
                            engine=inst.engine,
                            debug=inst.debug,
                            sync_info=mybir.SyncInfo(
                                on_wait=rest[j : j + 2], on_update=[]
                            ),
                        )
                    )
                inst.sync_info = mybir.SyncInfo(
                    on_wait=[waits[keep_idx]], on_update=list(si.on_update)
                )
                out.append(inst)
            if changed:
                blk.instructions = out


def _dedup_ldweights(nc: bass.Bass) -> None:
    """Drop PE weight reloads that reload the already-loaded stationary.

    tile_legalize splits every matmul into InstLdweights + a
    non-self-loading InstMatmult. Matmult does not clobber the PE weight
    array, so consecutive Ldweights with an identical weights AP are
    redundant — all but the first can go (saving ~100 ns of PE time each,
    ~21 us total here). A redundant Ldweights that carries semaphore
    waits/updates is replaced by an EventSemaphore on the same engine so
    the synchronization is preserved; any other PE instruction resets the
    tracked signature (conservative).
    """
    for fn in nc.m.functions:
        for blk in fn.blocks:
            out = []
            changed = False
            last_sig = None
            for inst in blk.instructions:
                if inst.opcode == "Matmult":
                    out.append(inst)
                    continue
                if inst.opcode != "Ldweights":
                    if inst.engine == mybir.EngineType.PE and inst.opcode not in (
                        "EventSemaphore",
                    ):
                        last_sig = None
                    out.append(inst)
                    continue
                a = inst.ins[0]
                sig = (a.memref, a.offset, str(a.ap), str(a.dtype))
                if sig != last_sig:
                    last_sig = sig
                    out.append(inst)
                    continue
                changed = True
                si = inst.sync_info
                waits = list(si.on_wait) if si is not None else []
                upds = list(si.on_update) if si is not None else []
                if waits or upds:
                    for j in range(0, max(len(waits), 1), 2):
                        out.append(
                            mybir.InstEventSemaphore(
                                name=f"{inst.name}-lw{j}",
                                opcode="EventSemaphore",
                                engine=inst.engine,
                                debug=inst.debug,
                                sync_info=mybir.SyncInfo(
                                    on_wait=waits[j : j + 2],
                                    on_update=upds if j == 0 else [],
                                ),
                            )
                        )
            if changed:
                blk.instructions = out


def _coef() -> np.ndarray:
    # [P, 2P] = [A | B] packed side by side (one SBUF tile, one DMA):
    #   A[k, m] = TAU^(m+128-k)                (cross-block band)
    #   B[k, m] = TAU^(m-k) for k <= m else 0  (triangular band)
    k = np.arange(2 * P)[:, None]
    m = np.arange(P)[None, :]
    e = m + P - k
    c = np.where(e >= 0, TAU ** np.maximum(e, 0).astype(np.float64), 0.0)
    return np.ascontiguousarray(
        np.hstack([c[:P], c[P:]]).astype(NP_DT)
    )


def _build() -> bass.Bass:
    nc = bass.Bass()
    xt = nc.dram_tensor("xt", [T, ROWS], MYBIR_DT, kind="ExternalInput")
    coef = nc.dram_tensor("coef", [P, 2 * P], MYBIR_DT, kind="ExternalInput")
    yt = nc.dram_tensor("yt", [T, ROWS], MYBIR_DT, kind="ExternalOutput")

    x_r = xt.rearrange("(i p) r -> i p r", p=P)   # 16 slabs [128, ROWS]
    y_r = yt.rearrange("(i p) r -> i p r", p=P)   # 16 blocks [128, ROWS]

    with TileContext(nc) as tc:
        with (
            tc.tile_pool(name="const", bufs=1) as cpool,
            tc.tile_pool(name="in", bufs=8) as ipool,
            tc.tile_pool(name="out", bufs=4) as opool,
            tc.tile_pool(name="psum", bufs=8, space="PSUM") as ppool,
        ):
            cf = cpool.tile([P, 2 * P], MYBIR_DT)
            nc.sync.dma_start(out=cf[:], in_=coef[:])
            cA = cf[:, 0:P]
            cB = cf[:, P : 2 * P]

            LAST = N_BLK - 1
            slabs = []
            for i in range(N_BLK):
                s = ipool.tile([P, ROWS], MYBIR_DT)
                if i == LAST:
                    # Final block: half-granular input and quarter-granular
                    # output so its writes are ready as the read stream ends
                    # (shortens the exposed tail chain).
                    h = ROWS // 2
                    nc.sync.dma_start(out=s[:, 0:h], in_=x_r[i][:, 0:h])
                    nc.sync.dma_start(out=s[:, h:ROWS], in_=x_r[i][:, h:ROWS])
                else:
                    nc.sync.dma_start(out=s[:], in_=x_r[i])
                slabs.append(s)

                utile = opool.tile([P, ROWS], MYBIR_DT)
                # All-A then all-B so the redundant-LDWEIGHTS dedup pass can
                # collapse each group to one weight load; the 8 chunks exactly
                # fill the 8 PSUM banks. Chunk direction alternates per block
                # so block i+1's A-matmuls only become ready (PSUM bank freed)
                # after block i's B-phase — keeping same-weight runs
                # contiguous in the scheduled PE order.
                order = list(range(N_CHUNK))
                if i % 2:
                    order.reverse()
                pts = {}
                for c in order:
                    pt = ppool.tile([P, CHUNK], mybir.dt.float32)
                    pts[c] = pt
                    sl = slice(c * CHUNK, (c + 1) * CHUNK)
                    if i > 0:
                        nc.tensor.matmul(
                            pt[:], lhsT=cA[:], rhs=slabs[i - 1][:, sl],
                            start=True, stop=False,
                        )
                copied = set()
                for c in order:
                    sl = slice(c * CHUNK, (c + 1) * CHUNK)
                    nc.tensor.matmul(
                        pts[c][:], lhsT=cB[:], rhs=slabs[i][:, sl],
                        start=(i == 0), stop=True,
                    )
                    if c % 2 == 0:
                        nc.vector.tensor_copy(utile[:, sl], pts[c][:])
                    else:
                        nc.scalar.copy(utile[:, sl], pts[c][:])
                    copied.add(c)
                    if i == LAST and (c ^ 1) in copied:
                        # final block streams output per chunk-pair so its
                        # writes are ready as the read stream ends
                        base = min(c, c ^ 1)
                        qs = slice(base * CHUNK, (base + 2) * CHUNK)
                        nc.scalar.dma_start(out=y_r[i][:, qs], in_=utile[:, qs])
                if i != LAST:
                    nc.scalar.dma_start(out=y_r[i], in_=utile[:])
                if i >= 1:
                    slabs[i - 1] = None

    _dedup_ldweights(nc)
    _split_excess_waits(nc)
    return nc


def kernel(x: np.ndarray, **_unused) -> np.ndarray:
    global _nc_cache, _coef_cache, last_results
    if _nc_cache is None:
        _nc_cache = _build()
        _coef_cache = _coef()
    nc = _nc_cache

    x = np.asarray(x)
    assert x.shape == (B, F, T), x.shape
    x16 = np.ascontiguousarray(x.reshape(N_CORES, ROWS, T), dtype=NP_DT)
    in_maps = [
        {"xt": np.ascontiguousarray(x16[c].T), "coef": _coef_cache}
        for c in range(N_CORES)
    ]
    last_results = run_bass_kernel_spmd(
        nc, in_maps, core_ids=list(range(N_CORES))
    )
    out = np.concatenate(
        [
            r["yt"].T.astype(np.float32).reshape(B_PER_CORE, F, T)
            for r in last_results.results
        ],
        axis=0,
    )
    return out
